# revision 1
# baseline (speedup 1.0000x reference)
"""Trainium2 Bass kernel for nn_DecorrelatedReNorm_17231408791729.

Math: the reference computes
    out = (X_c @ W @ W_inv + X_mean - running_mean) @ running_W
with W = U diag(S^-1/2) U^T and W_inv = U diag(S^1/2) U^T from eigh(cov).
W @ W_inv == I exactly (same eigenbasis), and X_c + X_mean == X, so
    out = (X - running_mean) @ running_W
identically; the eigh chain contributes only fp32 rounding (~1e-6 rel).

Strategy (data-parallel over N across 8 cores):
  - host: shard X rows 8 ways; transpose each shard to [C, rows] so the
    contraction dim (C) lands on SBUF partitions with contiguous DMAs;
    fold running_mean into a bias vector  b = -(running_mean @ running_W).
  - device (per core): for each 512-row macro-tile, stream X^T slab in,
    16 fp32 matmuls (K=4x128 chunks, N=512) accumulate in PSUM, DVE adds
    the broadcast bias while copying PSUM->SBUF, stream out.
  - host: concatenate the 8 row shards.
"""

import numpy as np
from contextlib import ExitStack

import concourse.bass as bass
import concourse.tile as tile
from concourse import bacc, mybir
from concourse.bass_utils import run_bass_kernel_spmd

C = 512
N_ROWS = 131072
N_CORES = 8
ROWS_PER_CORE = N_ROWS // N_CORES  # 16384
R_TILE = 512                       # rows per macro-tile
P = 128
KC = C // P                        # 4 contraction chunks
JT = R_TILE // P                   # 4 row sub-chunks per macro-tile


def build_bass(nrows: int = ROWS_PER_CORE):
    nc = bacc.Bacc(
        "TRN2",
        target_bir_lowering=False,
        debug=False,
        enable_asserts=False,
    )
    xt = nc.dram_tensor("xt", [C, nrows], mybir.dt.float32, kind="ExternalInput").ap()
    w = nc.dram_tensor("w", [C, C], mybir.dt.float32, kind="ExternalInput").ap()
    b = nc.dram_tensor("bias", [1, C], mybir.dt.float32, kind="ExternalInput").ap()
    out = nc.dram_tensor(
        "out", [nrows, C], mybir.dt.float32, kind="ExternalOutput"
    ).ap()

    t_count = nrows // R_TILE
    # [T, p, kc, r]: partition = cin within chunk, free = (chunk, row)
    xt_r = xt.rearrange("(kc p) (t r) -> t p kc r", p=P, r=R_TILE)
    # [p, kc, n]: partition = cin within chunk, free = (chunk, cout)
    w_r = w.rearrange("(kc p) n -> p kc n", p=P)
    # [T, p, j, n]: partition = row within sub-chunk, free = (sub-chunk, cout)
    out_r = out.rearrange("(t j p) n -> t p j n", j=JT, p=P)

    with tile.TileContext(nc) as tc, ExitStack() as ctx:
        singles = ctx.enter_context(tc.tile_pool(name="singles", bufs=1))
        xpool = ctx.enter_context(tc.tile_pool(name="x", bufs=3))
        opool = ctx.enter_context(tc.tile_pool(name="o", bufs=3))
        pspool = ctx.enter_context(tc.tile_pool(name="ps", bufs=8, space="PSUM"))

        w_tile = singles.tile([P, KC, C], mybir.dt.float32)
        nc.sync.dma_start(out=w_tile[:], in_=w_r)
        bias_tile = singles.tile([P, C], mybir.dt.float32)
        b_bcast = bass.AP(tensor=b.tensor, offset=b.offset, ap=[[0, P], [1, C]])
        nc.sync.dma_start(out=bias_tile[:], in_=b_bcast)

        for t in range(t_count):
            x_tile = xpool.tile([P, KC, R_TILE], mybir.dt.float32, tag="x")
            nc.sync.dma_start(out=x_tile[:], in_=xt_r[t])
            o_tile = opool.tile([P, JT, C], mybir.dt.float32, tag="o")
            for j in range(JT):
                ps = pspool.tile([P, C], mybir.dt.float32, tag="ps")
                for k in range(KC):
                    nc.tensor.matmul(
                        ps[:],
                        x_tile[:, k, bass.ts(j, P)],
                        w_tile[:, k, :],
                        start=(k == 0),
                        stop=(k == KC - 1),
                    )
                nc.vector.tensor_add(o_tile[:, j, :], ps[:], bias_tile[:])
            nc.sync.dma_start(out=out_r[t], in_=o_tile[:])

    nc.compile()
    return nc


_CACHE: dict = {}


def _prep_in_maps(X, running_mean, running_W):
    X = np.ascontiguousarray(np.asarray(X, dtype=np.float32))
    rm = np.asarray(running_mean, dtype=np.float32)
    rW = np.ascontiguousarray(np.asarray(running_W, dtype=np.float32))
    rows = X.shape[0] // N_CORES
    bias = (-(rm.astype(np.float64) @ rW.astype(np.float64))).astype(
        np.float32
    ).reshape(1, C)
    return [
        {
            "xt": np.ascontiguousarray(X[c * rows : (c + 1) * rows].T),
            "w": rW,
            "bias": bias,
        }
        for c in range(N_CORES)
    ]


def kernel(X, running_mean, running_W):
    in_maps = _prep_in_maps(X, running_mean, running_W)
    nc = _CACHE.get("nc")
    if nc is None:
        nc = build_bass()
        _CACHE["nc"] = nc
    res = run_bass_kernel_spmd(nc, in_maps, core_ids=list(range(N_CORES)))
    return np.concatenate([r["out"] for r in res.results], axis=0)


# revision 7
# speedup vs baseline: 4.1997x; 4.1997x over previous
"""Trainium2 Bass kernel for nn_DecorrelatedReNorm_17231408791729.

Math: the reference computes
    out = (X_c @ W @ W_inv + X_mean - running_mean) @ running_W
with W = U diag(S^-1/2) U^T and W_inv = U diag(S^1/2) U^T from eigh(cov).
W @ W_inv == I exactly (same eigenbasis), and X_c + X_mean == X, so
    out = (X - running_mean) @ running_W
identically; the eigh chain contributes only fp32 rounding (~1e-6 rel).

Strategy (data-parallel over N across 8 cores):
  - host: shard X rows 8 ways; transpose each shard to [C, rows] so the
    contraction dim (C) lands on SBUF partitions with contiguous DMAs;
    fold running_mean into a bias vector  b = -(running_mean @ running_W).
  - device (per core): for each 512-row macro-tile, stream X^T slab in,
    16 fp32 matmuls (K=4x128 chunks, N=512) accumulate in PSUM, DVE adds
    the broadcast bias while copying PSUM->SBUF, stream out.
  - host: concatenate the 8 row shards.
"""

import numpy as np
from contextlib import ExitStack

import concourse.bass as bass
import concourse.tile as tile
from concourse import bacc, mybir
from concourse.bass_utils import run_bass_kernel_spmd
from concourse.masks import make_identity

C = 512
N_ROWS = 131072
N_CORES = 8
ROWS_PER_CORE = N_ROWS // N_CORES  # 16384
R_TILE = 512                       # rows per macro-tile
P = 128
KC = C // P                        # 4 contraction chunks
JT = R_TILE // P                   # 4 row sub-chunks per macro-tile


def build_bass(nrows: int = ROWS_PER_CORE, mm_dt=None, reps: int = 1):
    mm_dt = mm_dt if mm_dt is not None else mybir.dt.float32
    nc = bacc.Bacc(
        "TRN2",
        target_bir_lowering=False,
        debug=False,
        enable_asserts=False,
    )
    xt = nc.dram_tensor("xt", [C, nrows], mm_dt, kind="ExternalInput").ap()
    w = nc.dram_tensor("w", [C, C], mm_dt, kind="ExternalInput").ap()
    b = nc.dram_tensor("bias", [1, C], mybir.dt.float32, kind="ExternalInput").ap()
    out = nc.dram_tensor(
        "out", [nrows, C], mybir.dt.float32, kind="ExternalOutput"
    ).ap()

    t_count = nrows // R_TILE
    # [T, p, kc, r]: partition = cin within chunk, free = (chunk, row)
    xt_r = xt.rearrange("(kc p) (t r) -> t p kc r", p=P, r=R_TILE)
    # [p, kc, n]: partition = cin within chunk, free = (chunk, cout)
    w_r = w.rearrange("(kc p) n -> p kc n", p=P)
    # [T, p, j, n]: partition = row within sub-chunk, free = (sub-chunk, cout)
    out_r = out.rearrange("(t j p) n -> t p j n", j=JT, p=P)

    with tile.TileContext(nc) as tc, ExitStack() as ctx:
        singles = ctx.enter_context(tc.tile_pool(name="singles", bufs=1))
        xpool = ctx.enter_context(tc.tile_pool(name="x", bufs=3))
        opool = ctx.enter_context(tc.tile_pool(name="o", bufs=3))
        pspool = ctx.enter_context(tc.tile_pool(name="ps", bufs=8, space="PSUM"))

        w_tile = singles.tile([P, KC, C], mm_dt)
        nc.sync.dma_start(out=w_tile[:], in_=w_r)
        bias_tile = singles.tile([P, C], mybir.dt.float32)
        b_bcast = bass.AP(tensor=b.tensor, offset=b.offset, ap=[[0, P], [1, C]])
        nc.sync.dma_start(out=bias_tile[:], in_=b_bcast)

        for _ in range(reps):
            for t in range(t_count):
                x_tile = xpool.tile([P, KC, R_TILE], mm_dt, tag="x")
                nc.sync.dma_start(out=x_tile[:], in_=xt_r[t])
                o_tile = opool.tile([P, JT, C], mybir.dt.float32, tag="o")
                for j in range(JT):
                    ps = pspool.tile([P, C], mybir.dt.float32, tag="ps")
                    for k in range(KC):
                        nc.tensor.matmul(
                            ps[:],
                            x_tile[:, k, bass.ts(j, P)],
                            w_tile[:, k, :],
                            start=(k == 0),
                            stop=(k == KC - 1),
                        )
                    nc.vector.tensor_add(o_tile[:, j, :], ps[:], bias_tile[:])
                nc.sync.dma_start(out=out_r[t], in_=o_tile[:])

    nc.compile()
    return nc


def build_bass_exact(nrows: int = ROWS_PER_CORE, reps: int = 1):
    """out = X + X @ (W - I) + bias, with the residual matmul in float32r.

    X rides the exact fp32 path (DVE add); the float32r truncation only
    touches the residual term, which is exactly zero when W == I. Input X
    is the natural [rows, C] layout; X^T tiles for the matmul are made
    on-chip with PE transposes.
    """
    f32, f32r = mybir.dt.float32, mybir.dt.float32r
    nc = bacc.Bacc(
        "TRN2",
        target_bir_lowering=False,
        debug=False,
        enable_asserts=False,
    )
    x = nc.dram_tensor("x", [nrows, C], f32, kind="ExternalInput").ap()
    r = nc.dram_tensor("r", [C, C], f32r, kind="ExternalInput").ap()
    b = nc.dram_tensor("bias", [1, C], f32, kind="ExternalInput").ap()
    out = nc.dram_tensor("out", [nrows, C], f32, kind="ExternalOutput").ap()

    t_count = nrows // R_TILE
    # [T, p, j, c]: partition = row within sub-chunk, free = (sub-chunk, col)
    x_r = x.rearrange("(t j p) c -> t p j c", j=JT, p=P)
    r_r = r.rearrange("(kc p) n -> p kc n", p=P)
    out_r = out.rearrange("(t j p) n -> t p j n", j=JT, p=P)

    with tile.TileContext(nc) as tc, ExitStack() as ctx:
        singles = ctx.enter_context(tc.tile_pool(name="singles", bufs=1))
        xpool = ctx.enter_context(tc.tile_pool(name="x", bufs=3))
        xtpool = ctx.enter_context(tc.tile_pool(name="xt", bufs=4))
        opool = ctx.enter_context(tc.tile_pool(name="o", bufs=3))
        pst_pool = ctx.enter_context(tc.tile_pool(name="pst", bufs=4, space="PSUM"))
        pso_pool = ctx.enter_context(tc.tile_pool(name="pso", bufs=4, space="PSUM"))

        r_tile = singles.tile([P, KC, C], f32r)
        nc.sync.dma_start(out=r_tile[:], in_=r_r)
        bias_tile = singles.tile([P, C], f32)
        b_bcast = bass.AP(tensor=b.tensor, offset=b.offset, ap=[[0, P], [1, C]])
        nc.sync.dma_start(out=bias_tile[:], in_=b_bcast)
        ident = singles.tile([P, P], f32)
        make_identity(nc, ident[:])

        for _ in range(reps):
            for t in range(t_count):
                x_tile = xpool.tile([P, JT, C], f32, tag="x")
                nc.sync.dma_start(out=x_tile[:], in_=x_r[t])
                o_tile = opool.tile([P, JT, C], f32, tag="o")
                for j in range(JT):
                    ps_t = pst_pool.tile([P, KC, P], f32, tag="pst")
                    for k in range(KC):
                        nc.tensor.transpose(
                            ps_t[:, k, :],
                            x_tile[:, j, bass.ts(k, P)],
                            ident[:],
                        )
                    # fp32 -> float32r rounding happens in this DVE copy
                    xT = xtpool.tile([P, KC, P], f32r, tag="xt")
                    nc.vector.tensor_copy(xT[:], ps_t[:])
                    ps_o = pso_pool.tile([P, C], f32, tag="pso")
                    for k in range(KC):
                        nc.tensor.matmul(
                            ps_o[:],
                            xT[:, k, :],
                            r_tile[:, k, :],
                            start=(k == 0),
                            stop=(k == KC - 1),
                        )
                    nc.vector.tensor_add(o_tile[:, j, :], ps_o[:], x_tile[:, j, :])
                    nc.gpsimd.tensor_add(o_tile[:, j, :], o_tile[:, j, :], bias_tile[:])
                nc.sync.dma_start(out=out_r[t], in_=o_tile[:])

    nc.compile()
    return nc


_CACHE: dict = {}


def _prep_in_maps(X, running_mean, running_W):
    """Inputs for build_bass (host-transposed X, full W)."""
    X = np.ascontiguousarray(np.asarray(X, dtype=np.float32))
    rm = np.asarray(running_mean, dtype=np.float32)
    rW = np.ascontiguousarray(np.asarray(running_W, dtype=np.float32))
    rows = X.shape[0] // N_CORES
    bias = (-(rm.astype(np.float64) @ rW.astype(np.float64))).astype(
        np.float32
    ).reshape(1, C)
    return [
        {
            "xt": np.ascontiguousarray(X[c * rows : (c + 1) * rows].T),
            "w": rW,
            "bias": bias,
        }
        for c in range(N_CORES)
    ]


def _prep_in_maps_exact(X, running_mean, running_W):
    """Inputs for build_bass_exact (natural-layout X shards, residual W - I)."""
    X = np.ascontiguousarray(np.asarray(X, dtype=np.float32))
    rm = np.asarray(running_mean, dtype=np.float32)
    rW = np.asarray(running_W, dtype=np.float32)
    rows = X.shape[0] // N_CORES
    r = np.ascontiguousarray(rW - np.eye(C, dtype=np.float32))
    bias = (-(rm.astype(np.float64) @ rW.astype(np.float64))).astype(
        np.float32
    ).reshape(1, C)
    return [
        {
            "x": np.ascontiguousarray(X[c * rows : (c + 1) * rows]),
            "r": r,
            "bias": bias,
        }
        for c in range(N_CORES)
    ]


def kernel(X, running_mean, running_W):
    in_maps = _prep_in_maps_exact(X, running_mean, running_W)
    nc = _CACHE.get("nc")
    if nc is None:
        nc = build_bass_exact()
        _CACHE["nc"] = nc
    res = run_bass_kernel_spmd(nc, in_maps, core_ids=list(range(N_CORES)))
    return np.concatenate([r["out"] for r in res.results], axis=0)
